# revision 6
# baseline (speedup 1.0000x reference)
"""Trainium2 Bass kernel for nn_Base_75265006895876 (retrieval_knn).

Data-parallel over batch B=128 -> 16 per core on 8 NeuronCores; the
cap_embedding table is replicated per core.  Per core, (t, b) pairs are
processed in groups of 8 pairs (= 128 gathered embedding rows):

  indirect-DMA gather rows (fp32 -> bf16 cast in DMA)
  -> 8x HWDGE xbar dma_start_transpose (SBUF->SBUF, D onto partitions)
  -> PE gram+dots matmuls (bf16, fp32 accum), fused masking matmuls
  -> cheap per-group column extractions into [128, NGROUPS] tiles
  -> one batched tail: norms / min-distance / cosine chains
  -> results tile -> single DMA out
"""

import sys

if "/opt/trn_rl_repo" not in sys.path:
    sys.path.insert(0, "/opt/trn_rl_repo")

import numpy as np

# ---- problem constants (hardcoded; kernel.py must be self-contained) ----
T, B, K, L, V, D = 17, 128, 16, 24, 30000, 1024
NCORES = 8
BL = B // NCORES              # 16 local batch rows per core
PAIRS = T * BL                # 272 (t, b) pairs per core
NG = PAIRS * K // 128         # 34 groups of 128 gathered rows
PPG = 128 // K                # 8 pairs per group
LPAD = 32                     # caption length padded 24 -> 32
NCAP = BL * LPAD // 128       # 4 caption gather groups
CH = D // 128                 # 8 contraction chunks of 128
CW = 128 + K                  # 144: chunk cols + sentence cols in tb
LARGE = 1.0e6

_CACHE = {}


def _build_nc():
    from concourse import bass, bacc, mybir

    f32 = mybir.dt.float32
    bf16 = mybir.dt.bfloat16
    AF = mybir.ActivationFunctionType
    ALU = mybir.AluOpType

    nc = bacc.Bacc("TRN2", debug=False)

    table = nc.dram_tensor("table", [V, D], f32, kind="ExternalInput")
    idx_topk_d = nc.dram_tensor("idx_topk", [128, NG], mybir.dt.int32,
                                kind="ExternalInput")
    idx_cap_d = nc.dram_tensor("idx_cap", [128, NCAP], mybir.dt.int32,
                               kind="ExternalInput")
    maskB_d = nc.dram_tensor("maskB", [128, NCAP * BL], bf16,
                             kind="ExternalInput")
    i128_d = nc.dram_tensor("i128", [128, 128], f32, kind="ExternalInput")
    i128b_d = nc.dram_tensor("i128b", [128, 128], bf16, kind="ExternalInput")
    ineg_d = nc.dram_tensor("ineg", [128, 128], bf16, kind="ExternalInput")
    cmask_d = nc.dram_tensor("cmask", [128, 128], bf16, kind="ExternalInput")
    w0_d = nc.dram_tensor("w0", [128, 128], f32, kind="ExternalInput")
    m0p_d = nc.dram_tensor("m0p", [128, NG * CH], f32, kind="ExternalInput")
    mdot_d = nc.dram_tensor("mdot", [128, NG * K], f32, kind="ExternalInput")

    res_d = nc.dram_tensor("res", [128, 3 * NG], f32, kind="ExternalOutput")

    from concourse.tile import TileContext
    from contextlib import ExitStack

    with ExitStack() as ctx:
        tc = ctx.enter_context(TileContext(nc))
        cp = ctx.enter_context(tc.tile_pool(name="cp", bufs=1))
        xp = ctx.enter_context(tc.tile_pool(name="xp", bufs=4))
        smp = ctx.enter_context(tc.tile_pool(name="smp", bufs=2))
        pwp = ctx.enter_context(tc.tile_pool(name="pwp", bufs=4, space="PSUM"))
        psp = ctx.enter_context(tc.tile_pool(name="psp", bufs=1, space="PSUM"))

        # ---- load constants / indices into SBUF ----
        c_i128 = cp.tile([128, 128], f32)
        nc.sync.dma_start(c_i128[:], i128_d[:])
        c_i128b = cp.tile([128, 128], bf16)
        nc.sync.dma_start(c_i128b[:], i128b_d[:])
        c_ineg = cp.tile([128, 128], bf16)
        nc.sync.dma_start(c_ineg[:], ineg_d[:])
        c_cm = cp.tile([128, 128], bf16)
        nc.sync.dma_start(c_cm[:], cmask_d[:])
        c_w0 = cp.tile([128, 128], f32)
        nc.sync.dma_start(c_w0[:], w0_d[:])
        c_m0p = cp.tile([128, NG * CH], f32)
        nc.sync.dma_start(c_m0p[:], m0p_d[:])
        c_mdot = cp.tile([128, NG * K], f32)
        nc.sync.dma_start(c_mdot[:], mdot_d[:])
        c_maskB = cp.tile([128, NCAP * BL], bf16)
        nc.sync.dma_start(c_maskB[:], maskB_d[:])
        c_idxt = cp.tile([128, NG], mybir.dt.int32)
        nc.sync.dma_start(c_idxt[:], idx_topk_d[:])
        c_idxc = cp.tile([128, NCAP], mybir.dt.int32)
        nc.sync.dma_start(c_idxc[:], idx_cap_d[:])

        res_sb = cp.tile([128, 3 * NG], f32)

        # per-group accumulation tiles for the batched tail
        sq_all = cp.tile([128, NG], f32)
        sqb_all = cp.tile([128, NG], bf16)
        mx_all = cp.tile([128, NG], f32)
        g0c_all = cp.tile([128, NG * CH], f32)
        dot_all = cp.tile([128, NG * K], f32)

        # ---- phase A: sentence embeddings ----
        sent_ps = psp.tile([16, 1024], f32, space="PSUM")
        for c in range(NCAP):
            cap = xp.tile([128, D], bf16, tag="xg", name=f"cap{c}")
            nc.gpsimd.indirect_dma_start(
                out=cap[:], out_offset=None, in_=table[:],
                in_offset=bass.IndirectOffsetOnAxis(
                    ap=c_idxc[:, c:c + 1], axis=0),
            )
            for hh in range(2):
                nc.tensor.matmul(
                    sent_ps[:, 512 * hh:512 * (hh + 1)],
                    lhsT=c_maskB[:, BL * c:BL * (c + 1)],
                    rhs=cap[:, 512 * hh:512 * (hh + 1)],
                    start=(c == 0), stop=(c == NCAP - 1),
                )
        sent_f = cp.tile([16, 1024], f32)
        nc.scalar.copy(sent_f[:], sent_ps[:])
        sq_scr = cp.tile([16, 1024], f32)
        ssq = cp.tile([16, 1], f32)
        nc.scalar.activation(sq_scr[:], sent_f[:], AF.Square,
                             accum_out=ssq[:])
        ssq2 = cp.tile([16, 1], f32)
        nc.vector.tensor_scalar_max(ssq2[:], ssq[:], 1e-16)
        rss = cp.tile([16, 1], f32)
        nc.vector.reciprocal(rss[:], ssq2[:])
        rsent = cp.tile([16, 1], f32)
        nc.scalar.sqrt(rsent[:], rss[:])
        sentnb = cp.tile([16, 1024], bf16)
        nc.vector.tensor_scalar_mul(sentnb[:], sent_f[:], rsent[:])

        # sentence columns -> both tb tiles via xbar transpose [16,128]->[128,16]
        tb_tiles = []
        for i in range(2):
            tbt = cp.tile([128, CH * CW], bf16, name=f"tb{i}")
            tb_tiles.append(tbt)
            for c in range(CH):
                eng = nc.sync if (c % 2 == 0) else nc.scalar
                eng.dma_start_transpose(
                    tbt[:, CW * c + 128:CW * (c + 1)],
                    sentnb[:16, 128 * c:128 * (c + 1)])

        # ---- phase B: 34 groups ----
        for g in range(NG):
            x = xp.tile([128, D], bf16, tag="xg", name=f"x{g}")
            nc.gpsimd.indirect_dma_start(
                out=x[:], out_offset=None, in_=table[:],
                in_offset=bass.IndirectOffsetOnAxis(
                    ap=c_idxt[:, g:g + 1], axis=0),
            )
            tb = tb_tiles[g % 2]
            for c in range(CH):
                eng = nc.sync if (c % 2 == 0) else nc.scalar
                eng.dma_start_transpose(
                    tb[:, CW * c:CW * c + 128],
                    x[:, 128 * c:128 * (c + 1)])

            wk = pwp.tile([128, 512], f32, space="PSUM", tag="wk",
                          name=f"wk{g}")
            # gram [*,0:128] and dots [*,128:144] in one matmul per chunk
            for c in range(CH):
                nc.tensor.matmul(
                    wk[:, 0:CW],
                    lhsT=tb[:, CW * c:CW * c + 128],
                    rhs=tb[:, CW * c:CW * c + CW],
                    start=(c == 0), stop=False,
                    skip_group_check=True,
                )
            # sq = diag(gram): DVE mask-mult, ACT accumulate-reduce
            scr128 = smp.tile([128, 128], f32, tag="scr128", name=f"s1{g}")
            scr128b = smp.tile([128, 128], f32, tag="scr128b", name=f"t1{g}")
            nc.vector.tensor_tensor(out=scr128[:], in0=wk[:, 0:128],
                                    in1=c_i128[:], op=ALU.mult)
            nc.scalar.activation(scr128b[:], scr128[:], AF.Copy,
                                 accum_out=sq_all[:, g:g + 1])
            nc.vector.tensor_copy(sqb_all[:, g:g + 1], sq_all[:, g:g + 1])
            # gram col-0-of-block (8 strided cols) and dots -> batch tiles
            gv = wk[:, 0:128].rearrange("p (a b) -> p a b", b=K)[:, :, 0:1]
            nc.scalar.copy(
                g0c_all[:, CH * g:CH * (g + 1)].rearrange(
                    "p (a b) -> p a b", b=1), gv)
            nc.scalar.copy(dot_all[:, K * g:K * (g + 1)], wk[:, 128:128 + K])
            # pollute gram psum: += -0.5*bf16(sq)[c] (broadcast) and
            # += -0.5*LARGE*(1 - blockdiag + I)
            nc.tensor.matmul(
                wk[:, 0:128], lhsT=sqb_all[:, g:g + 1].to_broadcast([128, 128]),
                rhs=c_ineg[:], start=False, stop=False,
                skip_group_check=True,
            )
            nc.tensor.matmul(
                wk[:, 0:128], lhsT=c_cm[:], rhs=c_i128b[:],
                start=False, stop=True, skip_group_check=True,
            )
            nc.vector.tensor_reduce(mx_all[:, g:g + 1], wk[:, 0:128],
                                    axis=mybir.AxisListType.X,
                                    op=ALU.max)

        # ---- batched tail over all 34 groups at once ----
        # y = 2*sq - bf16(sq)  (cancels the bf16-rounded broadcast exactly
        # for duplicate-row zero distances)
        sqbf = cp.tile([128, NG], f32)
        nc.vector.tensor_copy(sqbf[:], sqb_all[:])
        y_all = cp.tile([128, NG], f32)
        nc.vector.tensor_scalar(out=y_all[:], in0=sq_all[:], scalar1=2.0,
                                scalar2=None, op0=ALU.mult)
        nc.vector.tensor_tensor(out=y_all[:], in0=y_all[:], in1=sqbf[:],
                                op=ALU.subtract)
        # md2 = max(-2*mx + y, 1e-12); min_dist = sqrt(md2)
        md2 = cp.tile([128, NG], f32)
        nc.scalar.activation(md2[:], mx_all[:], AF.Copy, scale=-2.0)
        nc.vector.tensor_tensor(out=md2[:], in0=md2[:], in1=y_all[:],
                                op=ALU.add)
        nc.vector.tensor_scalar_max(md2[:], md2[:], 1e-12)
        nc.scalar.sqrt(res_sb[:, 0:NG], md2[:])
        # rn = 1/||E_r||, rn0 = rn[block row 0] via selection matmul
        rq = cp.tile([128, NG], f32)
        nc.vector.reciprocal(rq[:], sq_all[:])
        rn_all = cp.tile([128, NG], f32)
        nc.scalar.sqrt(rn_all[:], rq[:])
        rn0_ps = pwp.tile([128, 512], f32, space="PSUM", tag="wk",
                          name="rn0ps")
        nc.tensor.matmul(rn0_ps[:, 0:NG], lhsT=c_w0[:], rhs=rn_all[:],
                         start=True, stop=True)
        rn0_all = cp.tile([128, NG], f32)
        nc.scalar.copy(rn0_all[:], rn0_ps[:, 0:NG])
        # g0 = sum over strided cols (masked); cos = g0 * rn * rn0
        g0m = cp.tile([128, NG * CH], f32)
        nc.gpsimd.tensor_tensor(out=g0m[:], in0=g0c_all[:], in1=c_m0p[:],
                                op=ALU.mult)
        g0_all = cp.tile([128, NG], f32)
        nc.vector.tensor_reduce(
            g0_all[:], g0m[:].rearrange("p (g a) -> p g a", a=CH),
            axis=mybir.AxisListType.X, op=ALU.add)
        cosa = cp.tile([128, NG], f32)
        nc.vector.tensor_tensor(out=cosa[:], in0=g0_all[:], in1=rn_all[:],
                                op=ALU.mult)
        nc.vector.tensor_tensor(out=res_sb[:, NG:2 * NG], in0=cosa[:],
                                in1=rn0_all[:], op=ALU.mult)
        # sent_cos = (dots masked-extract) * rn
        dvm = cp.tile([128, NG * K], f32)
        nc.gpsimd.tensor_tensor(out=dvm[:], in0=dot_all[:], in1=c_mdot[:],
                                op=ALU.mult)
        dv_all = cp.tile([128, NG], f32)
        nc.vector.tensor_reduce(
            dv_all[:], dvm[:].rearrange("p (g a) -> p g a", a=K),
            axis=mybir.AxisListType.X, op=ALU.add)
        nc.vector.tensor_tensor(out=res_sb[:, 2 * NG:3 * NG], in0=dv_all[:],
                                in1=rn_all[:], op=ALU.mult)

        nc.sync.dma_start(res_d[:], res_sb[:])

    nc.compile()
    return nc


def _get_nc():
    if "nc" not in _CACHE:
        _CACHE["nc"] = _build_nc()
    return _CACHE["nc"]


# ---------------- host-side preparation ----------------

def _host_consts():
    import ml_dtypes
    f = np.float32
    bf = ml_dtypes.bfloat16
    i128 = np.eye(128, dtype=f)
    i128b = np.eye(128).astype(bf)
    ineg = (-0.5 * np.eye(128)).astype(bf)
    blk = np.kron(np.eye(PPG), np.ones((K, K))).astype(f)  # block diagonal
    cmask = (-0.5 * LARGE * (1.0 - blk + np.eye(128))).astype(bf)
    # w0[q, m] = 1 iff q == K*(m//K)
    w0 = np.zeros((128, 128), f)
    m = np.arange(128)
    w0[(m // K) * K, m] = 1.0
    # m0p[r, cb] = (cb == r//K) * (r % K != 0), replicated per group
    r = np.arange(128)
    m0p1 = np.zeros((128, CH), f)
    m0p1[r, r // K] = (r % K != 0).astype(f)
    m0p = np.tile(m0p1, (1, NG))
    # mdot_h[r, c] = 1 iff c == 8h + r//K with h = g % 2
    mdot = np.zeros((128, NG * K), f)
    for g in range(NG):
        hh = g % 2
        mdot[r, K * g + 8 * hh + r // K] = 1.0
    return i128, i128b, ineg, cmask, w0, m0p, mdot


def _core_inputs(topk, cap, cap_len, table_np):
    """Build the per-core in_maps for run_bass_kernel_spmd."""
    import ml_dtypes
    bf = ml_dtypes.bfloat16
    i128, i128b, ineg, cmask, w0, m0p, mdot = _host_consts()
    in_maps = []
    for m in range(NCORES):
        bsl = slice(m * BL, (m + 1) * BL)
        tk = topk[:, bsl, :].astype(np.int64)          # [T, BL, K]
        cp_ = cap[bsl].astype(np.int64)                # [BL, L]
        cl = cap_len[bsl].astype(np.int64)             # [BL]

        idx_flat = tk.reshape(-1).astype(np.int32)     # [T*BL*K] = NG*128
        idx_topk = np.ascontiguousarray(
            idx_flat.reshape(NG, 128).T).astype(np.int32)  # [128, NG]

        cap_pad = np.zeros((BL, LPAD), np.int32)
        cap_pad[:, :L] = cp_.astype(np.int32)
        idx_cap = np.ascontiguousarray(
            cap_pad.reshape(-1).reshape(NCAP, 128).T).astype(np.int32)

        # maskB[row, col]: chunk c rows = 32a + l (a in 0..3), col = BL*c + 4c + a
        maskB = np.zeros((128, NCAP * BL), np.float32)
        for c in range(NCAP):
            for a in range(128 // LPAD):
                b = (128 // LPAD) * c + a
                ll = np.arange(LPAD)
                maskB[LPAD * a + ll, BL * c + b] = (ll < cl[b]).astype(
                    np.float32)

        in_maps.append({
            "table": table_np,
            "idx_topk": idx_topk,
            "idx_cap": idx_cap,
            "maskB": maskB.astype(bf),
            "i128": i128, "i128b": i128b, "ineg": ineg, "cmask": cmask,
            "w0": w0, "m0p": m0p, "mdot": mdot,
        })
    return in_maps


def _postprocess(results):
    """results: list of 8 dicts with 'res' [128, 3*NG] -> 3 arrays [B, T, K]."""
    per_core = []
    for m in range(NCORES):
        res = np.asarray(results[m]["res"])            # [128, 3*NG]
        r5 = res.reshape(PPG, K, 3, NG)                # [p_ig, i, o, g]
        r5 = r5.transpose(2, 3, 0, 1)                  # [o, g, p_ig, i]
        r5 = r5.reshape(3, NG * PPG, K)                # [o, p, i], p = t*BL+b
        r5 = r5.reshape(3, T, BL, K)                   # [o, t, b_loc, i]
        per_core.append(r5)
    full = np.concatenate([pc[:, :, None, :, :] for pc in per_core],
                          axis=2)                      # [3, T, m, b_loc, K]
    full = full.reshape(3, T, B, K).transpose(0, 2, 1, 3)  # [3, B, T, K]
    return full[0], full[1], full[2]


def _run(in_maps, trace=False, **kwargs):
    from concourse.bass_utils import run_bass_kernel_spmd
    nc = _get_nc()
    return run_bass_kernel_spmd(
        nc, in_maps, core_ids=list(range(NCORES)), trace=trace, **kwargs)


def kernel(topk_words, caption, cap_len, cap_embedding, _trace=False):
    topk = np.asarray(topk_words)
    cap = np.asarray(caption)
    cl = np.asarray(cap_len)
    table_np = np.ascontiguousarray(np.asarray(cap_embedding,
                                               dtype=np.float32))
    in_maps = _core_inputs(topk, cap, cl, table_np)
    br = _run(in_maps, trace=_trace)
    out = _postprocess(br.results)
    if _trace:
        kernel.last_results = br
    return out


# revision 12
# speedup vs baseline: 3.4607x; 3.4607x over previous
"""Trainium2 Bass kernel for nn_Base_75265006895876 (retrieval_knn).

Data-parallel over batch B=128 -> 16 per core on 8 NeuronCores; the
cap_embedding table is replicated per core.  Per core, (t, b) pairs are
processed in groups of 8 pairs (= 128 gathered embedding rows):

  indirect-DMA gather rows (fp32 -> bf16 cast in DMA)
  -> 8x HWDGE xbar dma_start_transpose (SBUF->SBUF, D onto partitions)
  -> PE gram+dots matmuls (bf16, fp32 accum), fused masking matmuls
  -> cheap per-group column extractions into [128, NGROUPS] tiles
  -> one batched tail: norms / min-distance / cosine chains
  -> results tile -> single DMA out
"""

import sys

if "/opt/trn_rl_repo" not in sys.path:
    sys.path.insert(0, "/opt/trn_rl_repo")

import numpy as np

# ---- problem constants (hardcoded; kernel.py must be self-contained) ----
T, B, K, L, V, D = 17, 128, 16, 24, 30000, 1024
NCORES = 8
BL = B // NCORES              # 16 local batch rows per core
PAIRS = T * BL                # 272 (t, b) pairs per core
NG = PAIRS * K // 128         # 34 groups of 128 gathered rows
PPG = 128 // K                # 8 pairs per group
LPAD = 32                     # caption length padded 24 -> 32
NCAP = BL * LPAD // 128       # 4 caption gather groups
CH = D // 128                 # 8 contraction chunks of 128
CW = 128 + K                  # 144: chunk cols + sentence cols in tb
LARGE = 1.0e6

_CACHE = {}


def _build_nc():
    from concourse import bass, bacc, mybir

    f32 = mybir.dt.float32
    bf16 = mybir.dt.bfloat16
    AF = mybir.ActivationFunctionType
    ALU = mybir.AluOpType

    nc = bacc.Bacc("TRN2", debug=False)

    table = nc.dram_tensor("table", [V, D], f32, kind="ExternalInput")
    idx_topk_d = nc.dram_tensor("idx_topk", [128, NG], mybir.dt.int32,
                                kind="ExternalInput")
    idx_cap_d = nc.dram_tensor("idx_cap", [128, NCAP], mybir.dt.int32,
                               kind="ExternalInput")
    maskB_d = nc.dram_tensor("maskB", [128, NCAP * BL], bf16,
                             kind="ExternalInput")
    i128_d = nc.dram_tensor("i128", [128, 128], f32, kind="ExternalInput")
    i128b_d = nc.dram_tensor("i128b", [128, 128], bf16, kind="ExternalInput")
    ineg_d = nc.dram_tensor("ineg", [128, 128], bf16, kind="ExternalInput")
    cmask_d = nc.dram_tensor("cmask", [128, 128], bf16, kind="ExternalInput")
    w0_d = nc.dram_tensor("w0", [128, 128], f32, kind="ExternalInput")
    m0p_d = nc.dram_tensor("m0p", [128, NG * CH], f32, kind="ExternalInput")
    mdot_d = nc.dram_tensor("mdot", [128, NG * K], f32, kind="ExternalInput")

    res_d = nc.dram_tensor("res", [128, 3 * NG], f32, kind="ExternalOutput")

    from concourse.tile import TileContext
    from contextlib import ExitStack

    with ExitStack() as ctx:
        tc = ctx.enter_context(TileContext(nc))
        cp = ctx.enter_context(tc.tile_pool(name="cp", bufs=1))
        xp = ctx.enter_context(tc.tile_pool(name="xp", bufs=4))
        smp = ctx.enter_context(tc.tile_pool(name="smp", bufs=2))
        ptp = ctx.enter_context(tc.tile_pool(name="ptp", bufs=2, space="PSUM"))
        pwp = ctx.enter_context(tc.tile_pool(name="pwp", bufs=4, space="PSUM"))

        # ---- load constants / indices into SBUF ----
        c_i128 = cp.tile([128, 128], f32)
        nc.sync.dma_start(c_i128[:], i128_d[:])
        c_i128b = cp.tile([128, 128], bf16)
        nc.sync.dma_start(c_i128b[:], i128b_d[:])
        c_ineg = cp.tile([128, 128], bf16)
        nc.sync.dma_start(c_ineg[:], ineg_d[:])
        c_cm = cp.tile([128, 128], bf16)
        nc.sync.dma_start(c_cm[:], cmask_d[:])
        c_w0 = cp.tile([128, 128], f32)
        nc.sync.dma_start(c_w0[:], w0_d[:])
        c_m0p = cp.tile([128, NG * CH], f32)
        nc.sync.dma_start(c_m0p[:], m0p_d[:])
        c_mdot = cp.tile([128, NG * K], f32)
        nc.sync.dma_start(c_mdot[:], mdot_d[:])
        c_maskB = cp.tile([128, NCAP * BL], bf16)
        nc.sync.dma_start(c_maskB[:], maskB_d[:])
        c_idxt = cp.tile([128, NG], mybir.dt.int32)
        nc.sync.dma_start(c_idxt[:], idx_topk_d[:])
        c_idxc = cp.tile([128, NCAP], mybir.dt.int32)
        nc.sync.dma_start(c_idxc[:], idx_cap_d[:])

        res_sb = cp.tile([128, 3 * NG], f32)

        # per-group accumulation tiles for the batched tail
        sq_all = cp.tile([128, NG], f32)
        sqb_all = cp.tile([128, NG], bf16)
        mx_all = cp.tile([128, NG], f32)
        g0c_all = cp.tile([128, NG * CH], f32)
        dot_all = cp.tile([128, NG * K], f32)

        # ---- phase A: sentence embeddings ----
        sent_ps = ptp.tile([16, 1024], f32, space="PSUM", tag="sent", bufs=1)
        for c in range(NCAP):
            cap = xp.tile([128, D], bf16, tag="xg", name=f"cap{c}")
            nc.gpsimd.indirect_dma_start(
                out=cap[:], out_offset=None, in_=table[:],
                in_offset=bass.IndirectOffsetOnAxis(
                    ap=c_idxc[:, c:c + 1], axis=0),
            )
            for hh in range(2):
                nc.tensor.matmul(
                    sent_ps[:, 512 * hh:512 * (hh + 1)],
                    lhsT=c_maskB[:, BL * c:BL * (c + 1)],
                    rhs=cap[:, 512 * hh:512 * (hh + 1)],
                    start=(c == 0), stop=(c == NCAP - 1),
                )
        sent_f = cp.tile([16, 1024], f32)
        nc.scalar.copy(sent_f[:], sent_ps[:])
        sq_scr = cp.tile([16, 1024], f32)
        ssq = cp.tile([16, 1], f32)
        nc.scalar.activation(sq_scr[:], sent_f[:], AF.Square,
                             accum_out=ssq[:])
        ssq2 = cp.tile([16, 1], f32)
        nc.vector.tensor_scalar_max(ssq2[:], ssq[:], 1e-16)
        rss = cp.tile([16, 1], f32)
        nc.vector.reciprocal(rss[:], ssq2[:])
        rsent = cp.tile([16, 1], f32)
        nc.scalar.sqrt(rsent[:], rss[:])
        sentnb = cp.tile([16, 1024], bf16)
        nc.vector.tensor_scalar_mul(sentnb[:], sent_f[:], rsent[:])

        # sentence columns -> [128, 16] chunks via PE transpose, into both tb
        stp = pwp.tile([128, 256], bf16, space="PSUM", tag="wk", name="stp")
        for c in range(CH):
            nc.tensor.transpose(
                stp[:, K * c:K * (c + 1)],
                sentnb[:16, 128 * c:128 * (c + 1)],
                c_i128b[:16, :16],
            )
        tb_tiles = []
        for i in range(2):
            tbt = cp.tile([128, CH * CW], bf16, name=f"tb{i}")
            tb_tiles.append(tbt)
            dst = tbt[:].rearrange("p (c w) -> p c w", w=CW)[:, :, 128:128 + K]
            src = stp[:, 0:CH * K].rearrange("p (c w) -> p c w", w=K)
            nc.vector.tensor_copy(dst, src)

        # ---- phase B: 34 groups ----
        for g in range(NG):
            x = xp.tile([128, D], bf16, tag="xg", name=f"x{g}")
            nc.gpsimd.indirect_dma_start(
                out=x[:], out_offset=None, in_=table[:],
                in_offset=bass.IndirectOffsetOnAxis(
                    ap=c_idxt[:, g:g + 1], axis=0),
            )
            tp = ptp.tile([128, D], bf16, space="PSUM", tag="tp",
                          name=f"tp{g}")
            for c in range(CH):
                nc.tensor.transpose(
                    tp[:, 128 * c:128 * (c + 1)],
                    x[:, 128 * c:128 * (c + 1)],
                    c_i128b[:],
                )
            tb = tb_tiles[g % 2]
            # copy transposed chunks (downcast) into the CW-strided layout;
            # 3 chunks on DVE, 5 on ACT
            dst = tb[:].rearrange("p (c w) -> p c w", w=CW)[:, :, 0:128]
            src = tp[:].rearrange("p (c w) -> p c w", w=128)
            nc.vector.tensor_copy(dst[:, 0:3], src[:, 0:3])
            nc.scalar.copy(dst[:, 3:], src[:, 3:])

            wk = pwp.tile([128, 512], f32, space="PSUM", tag="wk",
                          name=f"wk{g}")
            # gram [*,0:128] and dots [*,128:144] in one matmul per chunk
            for c in range(CH):
                nc.tensor.matmul(
                    wk[:, 0:CW],
                    lhsT=tb[:, CW * c:CW * c + 128],
                    rhs=tb[:, CW * c:CW * c + CW],
                    start=(c == 0), stop=False,
                    skip_group_check=True,
                )
            # sq = diag(gram): DVE mask-mult, ACT accumulate-reduce
            scr128 = smp.tile([128, 128], f32, tag="scr128", name=f"s1{g}")
            scr128b = smp.tile([128, 128], f32, tag="scr128b", name=f"t1{g}")
            nc.vector.tensor_tensor(out=scr128[:], in0=wk[:, 0:128],
                                    in1=c_i128[:], op=ALU.mult)
            nc.scalar.activation(scr128b[:], scr128[:], AF.Copy,
                                 accum_out=sq_all[:, g:g + 1])
            nc.vector.tensor_copy(sqb_all[:, g:g + 1], sq_all[:, g:g + 1])
            # gram col-0-of-block (8 strided cols) and dots -> batch tiles
            gv = wk[:, 0:128].rearrange("p (a b) -> p a b", b=K)[:, :, 0:1]
            nc.scalar.copy(
                g0c_all[:, CH * g:CH * (g + 1)].rearrange(
                    "p (a b) -> p a b", b=1), gv)
            nc.scalar.copy(dot_all[:, K * g:K * (g + 1)], wk[:, 128:128 + K])
            # pollute gram psum: += -0.5*bf16(sq)[c] (broadcast) and
            # += -0.5*LARGE*(1 - blockdiag + I)
            nc.tensor.matmul(
                wk[:, 0:128], lhsT=sqb_all[:, g:g + 1].to_broadcast([128, 128]),
                rhs=c_ineg[:], start=False, stop=False,
                skip_group_check=True,
            )
            nc.tensor.matmul(
                wk[:, 0:128], lhsT=c_cm[:], rhs=c_i128b[:],
                start=False, stop=True, skip_group_check=True,
            )
            nc.vector.tensor_reduce(mx_all[:, g:g + 1], wk[:, 0:128],
                                    axis=mybir.AxisListType.X,
                                    op=ALU.max)

        # ---- batched tail over all 34 groups at once ----
        # y = 2*sq - bf16(sq)  (cancels the bf16-rounded broadcast exactly
        # for duplicate-row zero distances)
        sqbf = cp.tile([128, NG], f32)
        nc.vector.tensor_copy(sqbf[:], sqb_all[:])
        y_all = cp.tile([128, NG], f32)
        nc.vector.tensor_scalar(out=y_all[:], in0=sq_all[:], scalar1=2.0,
                                scalar2=None, op0=ALU.mult)
        nc.vector.tensor_tensor(out=y_all[:], in0=y_all[:], in1=sqbf[:],
                                op=ALU.subtract)
        # md2 = max(-2*mx + y, 1e-12); min_dist = sqrt(md2)
        md2 = cp.tile([128, NG], f32)
        nc.scalar.activation(md2[:], mx_all[:], AF.Copy, scale=-2.0)
        nc.vector.tensor_tensor(out=md2[:], in0=md2[:], in1=y_all[:],
                                op=ALU.add)
        nc.vector.tensor_scalar_max(md2[:], md2[:], 1e-12)
        nc.scalar.sqrt(res_sb[:, 0:NG], md2[:])
        # rn = 1/||E_r||, rn0 = rn[block row 0] via selection matmul
        rq = cp.tile([128, NG], f32)
        nc.vector.reciprocal(rq[:], sq_all[:])
        rn_all = cp.tile([128, NG], f32)
        nc.scalar.sqrt(rn_all[:], rq[:])
        rn0_ps = pwp.tile([128, 512], f32, space="PSUM", tag="wk",
                          name="rn0ps")
        nc.tensor.matmul(rn0_ps[:, 0:NG], lhsT=c_w0[:], rhs=rn_all[:],
                         start=True, stop=True)
        rn0_all = cp.tile([128, NG], f32)
        nc.scalar.copy(rn0_all[:], rn0_ps[:, 0:NG])
        # g0 = sum over strided cols (masked); cos = g0 * rn * rn0
        g0m = cp.tile([128, NG * CH], f32)
        nc.gpsimd.tensor_tensor(out=g0m[:], in0=g0c_all[:], in1=c_m0p[:],
                                op=ALU.mult)
        g0_all = cp.tile([128, NG], f32)
        nc.vector.tensor_reduce(
            g0_all[:], g0m[:].rearrange("p (g a) -> p g a", a=CH),
            axis=mybir.AxisListType.X, op=ALU.add)
        cosa = cp.tile([128, NG], f32)
        nc.vector.tensor_tensor(out=cosa[:], in0=g0_all[:], in1=rn_all[:],
                                op=ALU.mult)
        nc.vector.tensor_tensor(out=res_sb[:, NG:2 * NG], in0=cosa[:],
                                in1=rn0_all[:], op=ALU.mult)
        # sent_cos = (dots masked-extract) * rn
        dvm = cp.tile([128, NG * K], f32)
        nc.gpsimd.tensor_tensor(out=dvm[:], in0=dot_all[:], in1=c_mdot[:],
                                op=ALU.mult)
        dv_all = cp.tile([128, NG], f32)
        nc.vector.tensor_reduce(
            dv_all[:], dvm[:].rearrange("p (g a) -> p g a", a=K),
            axis=mybir.AxisListType.X, op=ALU.add)
        nc.vector.tensor_tensor(out=res_sb[:, 2 * NG:3 * NG], in0=dv_all[:],
                                in1=rn_all[:], op=ALU.mult)

        nc.sync.dma_start(res_d[:], res_sb[:])

    nc.compile()
    return nc


def _get_nc():
    if "nc" not in _CACHE:
        _CACHE["nc"] = _build_nc()
    return _CACHE["nc"]


# ---------------- host-side preparation ----------------

def _host_consts():
    import ml_dtypes
    f = np.float32
    bf = ml_dtypes.bfloat16
    i128 = np.eye(128, dtype=f)
    i128b = np.eye(128).astype(bf)
    ineg = (-0.5 * np.eye(128)).astype(bf)
    blk = np.kron(np.eye(PPG), np.ones((K, K))).astype(f)  # block diagonal
    cmask = (-0.5 * LARGE * (1.0 - blk + np.eye(128))).astype(bf)
    # w0[q, m] = 1 iff q == K*(m//K)
    w0 = np.zeros((128, 128), f)
    m = np.arange(128)
    w0[(m // K) * K, m] = 1.0
    # m0p[r, cb] = (cb == r//K) * (r % K != 0), replicated per group
    r = np.arange(128)
    m0p1 = np.zeros((128, CH), f)
    m0p1[r, r // K] = (r % K != 0).astype(f)
    m0p = np.tile(m0p1, (1, NG))
    # mdot_h[r, c] = 1 iff c == 8h + r//K with h = g % 2
    mdot = np.zeros((128, NG * K), f)
    for g in range(NG):
        hh = g % 2
        mdot[r, K * g + 8 * hh + r // K] = 1.0
    return i128, i128b, ineg, cmask, w0, m0p, mdot


def _core_inputs(topk, cap, cap_len, table_np):
    """Build the per-core in_maps for run_bass_kernel_spmd."""
    import ml_dtypes
    bf = ml_dtypes.bfloat16
    i128, i128b, ineg, cmask, w0, m0p, mdot = _host_consts()
    in_maps = []
    for m in range(NCORES):
        bsl = slice(m * BL, (m + 1) * BL)
        tk = topk[:, bsl, :].astype(np.int64)          # [T, BL, K]
        cp_ = cap[bsl].astype(np.int64)                # [BL, L]
        cl = cap_len[bsl].astype(np.int64)             # [BL]

        idx_flat = tk.reshape(-1).astype(np.int32)     # [T*BL*K] = NG*128
        idx_topk = np.ascontiguousarray(
            idx_flat.reshape(NG, 128).T).astype(np.int32)  # [128, NG]

        cap_pad = np.zeros((BL, LPAD), np.int32)
        cap_pad[:, :L] = cp_.astype(np.int32)
        idx_cap = np.ascontiguousarray(
            cap_pad.reshape(-1).reshape(NCAP, 128).T).astype(np.int32)

        # maskB[row, col]: chunk c rows = 32a + l (a in 0..3), col = BL*c + 4c + a
        maskB = np.zeros((128, NCAP * BL), np.float32)
        for c in range(NCAP):
            for a in range(128 // LPAD):
                b = (128 // LPAD) * c + a
                ll = np.arange(LPAD)
                maskB[LPAD * a + ll, BL * c + b] = (ll < cl[b]).astype(
                    np.float32)

        in_maps.append({
            "table": table_np,
            "idx_topk": idx_topk,
            "idx_cap": idx_cap,
            "maskB": maskB.astype(bf),
            "i128": i128, "i128b": i128b, "ineg": ineg, "cmask": cmask,
            "w0": w0, "m0p": m0p, "mdot": mdot,
        })
    return in_maps


def _postprocess(results):
    """results: list of 8 dicts with 'res' [128, 3*NG] -> 3 arrays [B, T, K]."""
    per_core = []
    for m in range(NCORES):
        res = np.asarray(results[m]["res"])            # [128, 3*NG]
        r5 = res.reshape(PPG, K, 3, NG)                # [p_ig, i, o, g]
        r5 = r5.transpose(2, 3, 0, 1)                  # [o, g, p_ig, i]
        r5 = r5.reshape(3, NG * PPG, K)                # [o, p, i], p = t*BL+b
        r5 = r5.reshape(3, T, BL, K)                   # [o, t, b_loc, i]
        per_core.append(r5)
    full = np.concatenate([pc[:, :, None, :, :] for pc in per_core],
                          axis=2)                      # [3, T, m, b_loc, K]
    full = full.reshape(3, T, B, K).transpose(0, 2, 1, 3)  # [3, B, T, K]
    return full[0], full[1], full[2]


def _run(in_maps, trace=False, **kwargs):
    from concourse.bass_utils import run_bass_kernel_spmd
    nc = _get_nc()
    return run_bass_kernel_spmd(
        nc, in_maps, core_ids=list(range(NCORES)), trace=trace, **kwargs)


def kernel(topk_words, caption, cap_len, cap_embedding, _trace=False):
    topk = np.asarray(topk_words)
    cap = np.asarray(caption)
    cl = np.asarray(cap_len)
    table_np = np.ascontiguousarray(np.asarray(cap_embedding,
                                               dtype=np.float32))
    in_maps = _core_inputs(topk, cap, cl, table_np)
    br = _run(in_maps, trace=_trace)
    out = _postprocess(br.results)
    if _trace:
        kernel.last_results = br
    return out
